# revision 31
# baseline (speedup 1.0000x reference)
"""AMSPNet (fixed iCOH) Trainium2 kernel — 8 NeuronCores, pure data parallel.

Mathematical simplification (verified to ~3e-6 vs the jax reference): for real
input the full-spectrum imaginary-coherence numerator cancels exactly, so
  * temporal adj  = sigmoid(0) = 0.5 everywhere -> GCN output identical across
    channels -> the (B*C) GRU collapses to a per-batch GRU driven by the
    channel-summed log-spectrum, and
  * spatial adj ~ 0 -> the whole MBFCA spatial branch is a constant computed
    from biases on the host.

Per-core device work (batch shard of 128):
  rFFT of 84 windows/elem as a DFT matmul -> log-amplitude -> channel sum ->
  5 sliding segments -> 2-matmul GCN -> 5-step GRU (315 hidden) -> fused head.
"""

import os
import sys
import numpy as np

for _p in ("/opt/trn_rl_repo",):
    if _p not in sys.path:
        sys.path.insert(0, _p)

B, C, D = 1024, 21, 1024
WIN, STRIDE, S = 256, 64, 5
OUT_GCN, OUT_GRU = 128, 315
N_CORES = 8
BS = B // N_CORES            # 128 batch elements per core
NW = D // WIN                # 4 windows per channel
HB = WIN // 2                # 128 freq bins kept
NROWS = BS * C * NW          # 10752 rows per core, row = c*512 + w*128 + b

# dtype knobs: FFT matmul dtype and GRU/GCN matmul dtype
FFT_MODE = os.environ.get("KERNEL_FFT_MODE", "fp16")   # fp16 | f32r | bf16
GRU_MODE = os.environ.get("KERNEL_GRU_MODE", "fp16")   # fp16 | bf16
TRACE = os.environ.get("KERNEL_TRACE", "0") == "1"
DMA_LATE = os.environ.get("KERNEL_DMA_LATE", "0") == "1"   # big weights on SWDGE
N_WARMUP = int(os.environ.get("KERNEL_WARMUP", "18"))
X_SWDGE = os.environ.get("KERNEL_X_SWDGE", "1") == "1"

_cache = {}


def _np_dt(mode):
    import ml_dtypes
    return {"f32r": np.float32, "fp16": np.float16, "bf16": ml_dtypes.bfloat16}[mode]


def _bir_dt(mode):
    import concourse.mybir as mybir
    return {"f32r": mybir.dt.float32r, "fp16": mybir.dt.float16,
            "bf16": mybir.dt.bfloat16}[mode]


def _build_nc():
    import concourse.bass as bass
    import concourse.bacc as bacc
    import concourse.mybir as mybir
    from concourse.tile import TileContext
    from contextlib import ExitStack

    dt = mybir.dt
    AF = mybir.ActivationFunctionType
    OP = mybir.AluOpType
    DT_X = _bir_dt(FFT_MODE)
    DT_G = _bir_dt(GRU_MODE)

    nc = bacc.Bacc()
    xt_d = nc.declare_dram_parameter("xt", [2, HB * NROWS], DT_X, isOutput=False)
    wfft_d = nc.declare_dram_parameter("wfft", [2 * HB, 2 * HB], DT_X, isOutput=False)
    wgcn_d = nc.declare_dram_parameter("wgcn", [WIN, OUT_GCN], DT_G, isOutput=False)
    wgru_d = nc.declare_dram_parameter("wgru", [512, 3 * OUT_GRU], DT_G, isOutput=False)
    wm1_d = nc.declare_dram_parameter("wm1", [384, 4], DT_G, isOutput=False)
    cons_d = nc.declare_dram_parameter("consts", [128, 6], dt.float32, isOutput=False)
    out_d = nc.declare_dram_parameter("out", [3, BS], dt.float32, isOutput=True)

    def xap(ap):
        return ap

    NBLK = C  # 21 blocks of 512 rows; block j == channel j

    with TileContext(nc) as tc, ExitStack() as ctx:
        const = ctx.enter_context(tc.tile_pool(name="const", bufs=1))
        xpool = ctx.enter_context(tc.tile_pool(name="x", bufs=3))
        SQ_BUFS = int(os.environ.get("KERNEL_SQ_BUFS", "3"))
        sqpool = ctx.enter_context(tc.tile_pool(name="sq", bufs=SQ_BUFS))

        # ---- persistent weights/constants ----
        # order: consts+wfft first (tiny), then x superblocks on the sync DGE;
        # bulky GRU weights go via the idle SWDGE.
        # DMA superblocks of 2048 rows (two 1024 compute blocks per DMA)
        DMA_BLOCKS = [(i * 2048, 2048) for i in range(5)] + [(10240, 512)]
        ROW_BLOCKS = [(i * 1024, 1024) for i in range(10)] + [(10240, 512)]
        x_tiles = {}
        cons_t = const.tile([128, 6], dt.float32, tag="cons")
        nc.sync.dma_start(cons_t[:], cons_d[:, :])
        wfft_t = []
        for k in range(2):
            t = const.tile([HB, 2 * HB], DT_X, tag=f"wfft{k}")
            nc.sync.dma_start(t[:], wfft_d[k * HB:(k + 1) * HB, :])
            wfft_t.append(t)

        def issue_xdma(di):
            off, bs = DMA_BLOCKS[di]
            x0 = xpool.tile([HB, 2048], DT_X, tag="x0")
            x1 = xpool.tile([HB, 2048], DT_X, tag="x1")
            eng = nc.gpsimd if X_SWDGE else nc.sync
            eng.dma_start(
                x0[:, 0:bs], bass.AP(xt_d.tensor if hasattr(xt_d, "tensor") else xt_d,
                                     off * HB, [[bs, HB], [1, bs]]))
            eng.dma_start(
                x1[:, 0:bs], bass.AP(xt_d.tensor if hasattr(xt_d, "tensor") else xt_d,
                                     HB * NROWS + off * HB, [[bs, HB], [1, bs]]))
            x_tiles[di] = (x0, x1)

        issue_xdma(0)
        KW = (128, 128, 128, 128)
        wgru_t = []
        ro = 0
        for k, kw in enumerate(KW):
            t = const.tile([kw, 3 * OUT_GRU], DT_G, tag=f"wgru{k}")
            (nc.gpsimd if DMA_LATE else nc.sync).dma_start(t[:], wgru_d[ro:ro + kw, :])
            wgru_t.append(t)
            ro += kw
        wgcn_t = []
        for k in range(2):
            t = const.tile([128, OUT_GCN], DT_G, tag=f"wgcn{k}")
            (nc.gpsimd if DMA_LATE else nc.sync).dma_start(t[:], wgcn_d[k * 128:(k + 1) * 128, :])
            wgcn_t.append(t)
        KH = (128, 128, 128)
        wm1_t = []
        ro = 0
        for k, kw in enumerate(KH):
            t = const.tile([kw, 4], DT_G, tag=f"wm1{k}")
            (nc.gpsimd if DMA_LATE else nc.sync).dma_start(t[:], wm1_d[ro:ro + kw, :])
            wm1_t.append(t)
            ro += kw

        # channel product accumulators: sum_c ln(amp2_c) == ln(prod_c amp2_c)
        chprod_a = const.tile([HB, 4 * BS], dt.float32, tag="chprod_a")
        chprod_b = const.tile([HB, 4 * BS], dt.float32, tag="chprod_b")
        nc.vector.memset(chprod_a[:], 1.0)
        nc.vector.memset(chprod_b[:], 1.0)

        # ---- FFT + log-amp + channel-sum: 1024-row blocks (2 channels) ----
        with tc.tile_pool(name="fftps", bufs=2, space="PSUM") as pspool:
            # HAM warmup: ~4us of dummy matmuls on the wfft tile while the
            # first x block is still in flight, so real FFT MMs start warm.
            wu_ps = pspool.tile([HB, 1024], dt.float32, tag="re")
            for _ in range(N_WARMUP):
                nc.tensor.matmul(wu_ps[:, 0:HB], wfft_t[0][:, 0:HB],
                                 wfft_t[0][:, 0:HB], start=True, stop=True)
            for j, (off, bs) in enumerate(ROW_BLOCKS):
                di, sub = j // 2, (j % 2) * 1024
                if di not in x_tiles:
                    issue_xdma(di)
                if di + 1 < len(DMA_BLOCKS) and di + 1 not in x_tiles:
                    issue_xdma(di + 1)
                xx0, xx1 = x_tiles[di]
                x0v = xx0[:, sub:sub + bs]
                x1v = xx1[:, sub:sub + bs]
                ps_re = pspool.tile([HB, 1024], dt.float32, tag="re")
                ps_im = pspool.tile([HB, 1024], dt.float32, tag="im")
                for h0 in range(0, bs, 512):
                    for kc in range(2):
                        xk = (x0v, x1v)[kc]
                        nc.tensor.matmul(ps_re[:, h0:h0 + 512],
                                         xap(wfft_t[kc][:, 0:HB]),
                                         xap(xk[:, h0:h0 + 512]),
                                         start=(kc == 0), stop=(kc == 1))
                        nc.tensor.matmul(ps_im[:, h0:h0 + 512],
                                         xap(wfft_t[kc][:, HB:2 * HB]),
                                         xap(xk[:, h0:h0 + 512]),
                                         start=(kc == 0), stop=(kc == 1))
                sq0 = sqpool.tile([HB, 1024], dt.bfloat16, tag="sq0")
                sq1 = sqpool.tile([HB, 1024], dt.bfloat16, tag="sq1")
                amp2 = sqpool.tile([HB, 1024], dt.bfloat16, tag="amp")
                zb = cons_t[0:HB, 5:6]
                nc.scalar.activation(sq0[:, 0:bs], ps_re[:, 0:bs], AF.Square, bias=zb)
                nc.scalar.activation(sq1[:, 0:bs], ps_im[:, 0:bs], AF.Square, bias=zb)
                nc.vector.tensor_tensor(amp2[:, 0:bs], sq0[:, 0:bs], sq1[:, 0:bs],
                                        op=OP.add)
                for h0 in range(0, bs, 512):
                    acc = chprod_a if h0 == 0 else chprod_b
                    nc.vector.tensor_tensor(acc[:], acc[:],
                                            amp2[:, h0:h0 + 512], op=OP.mult)

        # ---- ln(chprod) -> fp16, segment tiles, GCN matmuls ----
        nc.vector.tensor_tensor(chprod_a[:], chprod_a[:], chprod_b[:], op=OP.mult)
        cb = const.tile([HB, 4 * BS], DT_G, tag="cb")
        nc.scalar.activation(cb[:], chprod_a[:], AF.Ln, bias=cons_t[0:HB, 4:5])
        seg_chunks = {}
        for t in (0, 2, 4):
            w = t // 2
            seg_chunks[t] = (cb[:, w * BS:(w + 1) * BS], cb[:, (w + 1) * BS:(w + 2) * BS])
        for t in (1, 3):
            chunks = []
            for kc in range(2):
                st = const.tile([HB, BS], DT_G, tag=f"seg{t}{kc}")
                w_lo = (t * 64 + kc * 128) // 128
                nc.vector.tensor_copy(st[0:64, :], cb[64:128, w_lo * BS:(w_lo + 1) * BS])
                nc.vector.tensor_copy(st[64:128, :], cb[0:64, (w_lo + 1) * BS:(w_lo + 2) * BS])
                chunks.append(st[:, :])
            seg_chunks[t] = tuple(chunks)

        g_all = const.tile([OUT_GCN, S * BS], DT_G, tag="g_all")
        with tc.tile_pool(name="gcnps", bufs=1, space="PSUM") as gpsp:
            # separate banks per t-pair so the bias ACT overlaps later matmuls
            g_psA = gpsp.tile([OUT_GCN, 2 * BS], dt.float32, tag="gpsA")
            g_psB = gpsp.tile([OUT_GCN, 2 * BS], dt.float32, tag="gpsB")
            g_psC = gpsp.tile([OUT_GCN, BS], dt.float32, tag="gpsC")
            g_ps = [g_psA, g_psB, g_psC]
            for t in range(S):
                dst = g_ps[t // 2][:, (t % 2) * BS:(t % 2) * BS + BS]
                r0, r1 = seg_chunks[t]
                nc.tensor.matmul(dst, wgcn_t[0][:, :], r0, start=True, stop=False)
                nc.tensor.matmul(dst, wgcn_t[1][:, :], r1, start=False, stop=True)
                if t % 2 == 1 or t == 4:
                    pair = t // 2
                    w = 2 * BS if t % 2 == 1 else BS
                    nc.scalar.activation(g_all[:, pair * 2 * BS:pair * 2 * BS + w],
                                         g_ps[pair][:, 0:w],
                                         AF.Identity, bias=cons_t[:, 0:1])

        # ---- GRU state tiles (feature-chunk-major: free = chunk*128 + b) ----
        # chunk2 holds features 256..314 in partitions 0..58; partition 59 of
        # free-block 2 is a persistent ones-row used as the bias input row.
        h = const.tile([128, 3 * BS], DT_G, tag="h")
        rh = const.tile([128, 3 * BS], DT_G, tag="rh")
        r_t = const.tile([128, 3 * BS], DT_G, tag="r_t")
        u_t = const.tile([128, 3 * BS], DT_G, tag="u_t")
        c_t = const.tile([128, 3 * BS], DT_G, tag="c_t")
        e1 = const.tile([128, 3 * BS], DT_G, tag="e1")
        t2 = const.tile([128, 3 * BS], DT_G, tag="t2")
        for tile_ in (h, rh, r_t, u_t, c_t, e1, t2):
            nc.vector.memset(tile_[:], 0.0)
        nc.vector.memset(h[96:97, 2 * BS:3 * BS], 1.0)
        nc.vector.memset(rh[96:97, 2 * BS:3 * BS], 1.0)

        MW = (128, 128, 59)

        def gate_m_mms(ps_m, gate, src, t, m):
            rhs = [g_all[:, t * BS:(t + 1) * BS], src[0:128, 0:BS],
                   src[0:128, BS:2 * BS], src[0:128, 2 * BS:3 * BS]]
            mw = MW[m]
            col = gate * OUT_GRU + m * 128
            for k in range(4):
                kw = KW[k]
                nc.tensor.matmul(ps_m[0:mw, :],
                                 wgru_t[k][0:kw, col:col + mw], rhs[k],
                                 start=(k == 0), stop=(k == 3))



        with tc.tile_pool(name="grups", bufs=2, space="PSUM") as gps:
            zb5 = cons_t[0:128, 5:6]
            for t in range(S):
                # r gate, pipelined at m-tile granularity (separate psum banks
                # so ACT can read m-tile i while the PE accumulates m-tile i+1)
                for m in range(3):
                    mw = MW[m]
                    ps = gps.tile([128, BS], dt.float32, tag=f"gm{m}")
                    gate_m_mms(ps, 0, h, t, m)
                    nc.scalar.activation(r_t[:, m * BS:(m + 1) * BS], ps[:, :],
                                         AF.Sigmoid, bias=zb5)
                    if m < 2:
                        nc.vector.tensor_tensor(rh[:, m * BS:(m + 1) * BS],
                                                r_t[:, m * BS:(m + 1) * BS],
                                                h[:, m * BS:(m + 1) * BS], op=OP.mult)
                    else:
                        nc.vector.tensor_tensor(rh[0:59, 2 * BS:3 * BS],
                                                r_t[0:59, 2 * BS:3 * BS],
                                                h[0:59, 2 * BS:3 * BS], op=OP.mult)
                for m in range(3):
                    ps = gps.tile([128, BS], dt.float32, tag=f"gm{m}")
                    gate_m_mms(ps, 1, h, t, m)
                    nc.scalar.activation(u_t[:, m * BS:(m + 1) * BS], ps[:, :],
                                         AF.Sigmoid, bias=zb5)
                for m in range(3):
                    ps = gps.tile([128, BS], dt.float32, tag=f"gm{m}")
                    gate_m_mms(ps, 2, rh, t, m)
                    nc.scalar.activation(c_t[:, m * BS:(m + 1) * BS], ps[:, :],
                                         AF.Tanh, bias=zb5)
                # h' = u*h - (u-1)*c  (== u*h + (1-u)*c)
                nc.vector.tensor_tensor(e1[:], u_t[:], h[:], op=OP.mult)
                nc.vector.scalar_tensor_tensor(t2[:], u_t[:], 1.0, c_t[:],
                                               op0=OP.subtract, op1=OP.mult)
                nc.vector.tensor_tensor(h[:, 0:2 * BS], e1[:, 0:2 * BS],
                                        t2[:, 0:2 * BS], op=OP.subtract)
                nc.vector.tensor_tensor(h[0:59, 2 * BS:3 * BS], e1[0:59, 2 * BS:3 * BS],
                                        t2[0:59, 2 * BS:3 * BS], op=OP.subtract)

        # ---- fused head ----
        with tc.tile_pool(name="headps", bufs=1, space="PSUM") as hps:
            ps_z = hps.tile([3, BS], dt.float32, tag="headz")
            ps_s = hps.tile([1, BS], dt.float32, tag="heads")
            rhs_h = [h[0:128, 0:BS], h[0:128, BS:2 * BS], h[0:128, 2 * BS:3 * BS]]
            for k in range(3):
                nc.tensor.matmul(ps_z[:], wm1_t[k][:, 0:3], rhs_h[k],
                                 start=(k == 0), stop=(k == 2))
            for k in range(3):
                nc.tensor.matmul(ps_s[:], wm1_t[k][:, 3:4], rhs_h[k],
                                 start=(k == 0), stop=(k == 2))
            w0 = const.tile([1, BS], dt.float32, tag="w0")
            nc.scalar.activation(w0[:], ps_s[0:1, :], AF.Sigmoid, bias=cons_t[0:1, 3:4])
            ones3 = const.tile([1, 3], dt.float32, tag="ones3")
            nc.vector.memset(ones3[:], 1.0)
            ps_w3 = hps.tile([3, BS], dt.float32, tag="headw")
            nc.tensor.matmul(ps_w3[:], ones3[0:1, 0:3], w0[0:1, :],
                             start=True, stop=True)
            t1h = const.tile([3, BS], dt.float32, tag="t1h")
            nc.scalar.activation(t1h[:], ps_z[0:3, :], AF.Identity, bias=cons_t[0:3, 1:2])
            t2h = const.tile([3, BS], dt.float32, tag="t2h")
            nc.vector.tensor_tensor(t2h[:], t1h[:], ps_w3[0:3, :], op=OP.mult)
            outf = const.tile([3, BS], dt.float32, tag="outf")
            nc.scalar.activation(outf[:], t2h[:], AF.Identity, bias=cons_t[0:3, 2:3])
            nc.sync.dma_start(out_d[:, :], outf[:])

    nc.compile()
    return nc


def _host_prep(inp):
    """Precompute weight transforms shared across cores."""
    f32 = np.float32
    k = np.arange(2 * HB, dtype=np.float64)[:, None]
    m = np.arange(HB, dtype=np.float64)[None, :]
    ang = 2.0 * np.pi * k * m / (2 * HB)
    wfft = np.concatenate([np.cos(ang), -np.sin(ang)], 1).astype(f32)   # (256, 256)
    wgcn = (0.25 * np.asarray(inp["gcn_W"], f32))                        # (256, 128)
    gcnb = (10.5 * np.asarray(inp["gcn_b"], f32))                        # (128,)
    spa = np.tile(np.asarray(inp["mbfca_fc_b"], f32), 105)               # (315,)
    M1 = np.asarray(inp["fc1_W"], f32) @ np.asarray(inp["fc2_W"], f32)   # (315, 3)
    bias2 = np.asarray(inp["fc1_b"], f32) @ np.asarray(inp["fc2_W"], f32) \
        + np.asarray(inp["fc2_b"], f32)                                  # (3,)
    zs = spa @ M1
    c1 = float(spa @ np.asarray(inp["att_w_spa"], f32))
    wm1 = np.zeros((384, 4), f32)
    wm1[0:OUT_GRU, 0:3] = M1
    wm1[0:OUT_GRU, 3] = np.asarray(inp["att_w_tem"], f32)
    def aug(W, b):
        # (512, 315): rows 0..442 = W (rows 384..442 live in chunk3 rows 0..58),
        # chunk3 row 96 (global 480) = bias, all other pad rows zero
        out = np.zeros((512, OUT_GRU), f32)
        out[0:443] = np.asarray(W, f32)
        out[384 + 96] = np.asarray(b, f32)
        return out
    wgru = np.concatenate(
        [aug(inp["W_r"], inp["b_r"]), aug(inp["W_u"], inp["b_u"]),
         aug(inp["W_c"], inp["b_c"])], 1)                                # (512, 945)
    consts = np.zeros((128, 6), f32)
    consts[:, 0] = gcnb
    consts[0:3, 1] = -zs
    consts[0:3, 2] = zs + bias2
    consts[:, 3] = -c1
    consts[:, 4] = 1e-30
    consts[:, 5] = 0.0
    return dict(wfft=wfft, wgcn=wgcn, wgru=wgru, wm1=wm1, consts=consts)


def kernel(**inputs):
    from concourse.bass_utils import run_bass_kernel_spmd

    key = (FFT_MODE, GRU_MODE, DMA_LATE, N_WARMUP, X_SWDGE,
           os.environ.get("KERNEL_SQ_BUFS", "3"))
    if key not in _cache:
        _cache[key] = _build_nc()
    nc = _cache[key]

    p = _host_prep(inputs)
    dt_x = _np_dt(FFT_MODE)
    dt_g = _np_dt(GRU_MODE)
    shared = {
        "wfft": np.ascontiguousarray(p["wfft"].astype(dt_x)),
        "wgcn": np.ascontiguousarray(p["wgcn"].astype(dt_g)),
        "wgru": np.ascontiguousarray(p["wgru"].astype(dt_g)),
        "wm1": np.ascontiguousarray(p["wm1"].astype(dt_g)),
        "consts": np.ascontiguousarray(p["consts"]),
    }
    xs = np.asarray(inputs["x"], np.float32)[:, 0]     # (B, C, D)
    ROW_BLOCKS = [(i * 2048, 2048) for i in range(5)] + [(10240, 512)]
    in_maps = []
    for i in range(N_CORES):
        sh = xs[i * BS:(i + 1) * BS]                    # (BS, C, D)
        xt = sh.reshape(BS, C, NW, 2, HB).transpose(3, 4, 1, 2, 0).reshape(
            2, HB, NROWS).astype(dt_x)
        flat = np.empty((2, HB * NROWS), dt_x)
        for kc in range(2):
            pos = 0
            for (off, bs) in ROW_BLOCKS:
                flat[kc, pos:pos + HB * bs] = xt[kc, :, off:off + bs].ravel()
                pos += HB * bs
        in_maps.append({"xt": flat, **shared})

    res = run_bass_kernel_spmd(nc, in_maps, core_ids=list(range(N_CORES)),
                               trace=TRACE)
    if TRACE and res.exec_time_ns is not None:
        print(f"HW exec time: {res.exec_time_ns} ns")
    out = np.concatenate([r["out"].T for r in res.results], 0)  # (B, 3)
    return out.astype(np.float32)
